# revision 2
# baseline (speedup 1.0000x reference)
"""Trainium2 Bass kernel for nn_KKLayer (spectral channel-mix layer).

Math identity: the reference computes
    y = Re(IFFT2((A + iB) . conj(FFT2(x))))            (channel mix in freq domain)
Since channel mixing commutes with the spatial FFT and, for real x,
IFFT2(conj(FFT2(x))) is x spatially "negated" (h -> (-h) mod H, w -> (-w) mod W),
the whole layer collapses to
    y[b,o,h,w] = sum_i A[o,i] * x[b,i,(H-h)%H,(W-w)%W]
(betas drop out of the real part entirely).

Kernel: data-parallel over batch (8 batches -> 8 cores). The spatial flip is a
pure layout permutation, applied on the host while converting x to fp16, so
every device access is contiguous. Device I/O is fp16 (rel-err gate is 2e-2;
measured ~5e-4), halving HBM traffic vs fp32: 4MB in + 4MB out per core
(~23.5us at the 358 GB/s per-core HBM cap) instead of 16MB (~47us).

Per core:
  - load alphas^T (fp16 stationary weights) + x[b] (fp16, pre-flipped) in 8
    contiguous 512KB chunks
  - per chunk: 4 matmuls [K=128, M=128, N=512] fp16 -> fp32 PSUM
  - PSUM->SBUF copies cast fp32 -> fp16 (2 on DVE + 2 on ACT per chunk)
  - contiguous 512KB fp16 DMA-out per chunk
"""

import numpy as np

import concourse.bass as bass
import concourse.bacc as bacc
import concourse.mybir as mybir
from concourse import tile
from concourse.bass_utils import run_bass_kernel_spmd

B, CIN, COUT, H, W = 8, 128, 128, 128, 128
HW = H * W              # 16384
N_CORES = 8
CHUNK = 2048            # columns per DMA chunk (4KB/partition in fp16)
NCH = HW // CHUNK       # 8 chunks
MMN = 512               # matmul free dim (one PSUM bank of fp32)
MM_PER_CH = CHUNK // MMN

F16 = mybir.dt.float16
F32 = mybir.dt.float32

# dest row h reads source row (H - h) % H
_FLIP = (-np.arange(H)) % H


def _build_nc():
    nc = bacc.Bacc(None, target_bir_lowering=False)
    x = nc.dram_tensor("x", [CIN, HW], F16, kind="ExternalInput")
    wT = nc.dram_tensor("wT", [CIN, COUT], F16, kind="ExternalInput")
    y = nc.dram_tensor("y", [COUT, HW], F16, kind="ExternalOutput")

    with tile.TileContext(nc) as tc:
        with (
            tc.tile_pool(name="wp", bufs=1) as wpool,
            tc.tile_pool(name="xp", bufs=1) as xpool,
            tc.tile_pool(name="yp", bufs=1) as ypool,
            tc.tile_pool(name="ps", bufs=8, space="PSUM") as pspool,
        ):
            w_t = wpool.tile([CIN, COUT], F16)
            nc.sync.dma_start(w_t[:], wT[:])

            # all input DMAs issued first: no waits, so they stream
            # back-to-back on the sync HWDGE ring
            xch = []
            for k in range(NCH):
                t = xpool.tile([CIN, CHUNK], F16, tag=f"x{k}", name=f"xch{k}")
                nc.sync.dma_start(t[:], x[:, CHUNK * k:CHUNK * (k + 1)])
                xch.append(t)

            ych = [
                ypool.tile([COUT, CHUNK], F16, tag=f"y{k}", name=f"ych{k}")
                for k in range(NCH)
            ]

            for k in range(NCH):
                for j in range(MM_PER_CH):
                    ps = pspool.tile(
                        [COUT, MMN], F32, tag="ps", name=f"ps{k}_{j}"
                    )
                    nc.tensor.matmul(
                        ps[:],
                        w_t[:],
                        xch[k][:, MMN * j:MMN * (j + 1)],
                        start=True,
                        stop=True,
                    )
                    dst = ych[k][:, MMN * j:MMN * (j + 1)]
                    # split evacuation DVE/ACT so neither lags the DMA cadence
                    if j < 2:
                        nc.vector.tensor_copy(dst, ps[:])
                    else:
                        nc.scalar.copy(dst, ps[:])
                nc.sync.dma_start(y[:, CHUNK * k:CHUNK * (k + 1)], ych[k][:])
    nc.compile()
    return nc


_NC_CACHE = {}


def _get_nc():
    if "nc" not in _NC_CACHE:
        _NC_CACHE["nc"] = _build_nc()
    return _NC_CACHE["nc"]


def _prep_in_maps(x, alphas):
    """host prep: spatial flip + fp16 cast (layout/precision only, no math)"""
    x16 = np.asarray(x).astype(np.float16)
    xf = x16[:, :, _FLIP[:, None], _FLIP[None, :]]  # [B, CIN, H, W]
    wT = np.ascontiguousarray(
        np.asarray(alphas, dtype=np.float32).T
    ).astype(np.float16)
    return [
        {"x": np.ascontiguousarray(xf[c].reshape(CIN, HW)), "wT": wT}
        for c in range(N_CORES)
    ]


def kernel(x, alphas, betas=None, **_unused):
    nc = _get_nc()
    in_maps = _prep_in_maps(x, alphas)
    res = run_bass_kernel_spmd(nc, in_maps, core_ids=list(range(N_CORES)))
    out = np.stack(
        [
            np.asarray(res.results[c]["y"], dtype=np.float32).reshape(
                COUT, H, W
            )
            for c in range(N_CORES)
        ]
    )
    return out
